# revision 12
# baseline (speedup 1.0000x reference)
"""Distributed Trainium2 Bass kernel for nn_DiffusedAttention (GNN message passing).

Strategy:
- Nodes sharded 6250/core across 8 NeuronCores; per-core nodes permuted by
  in-degree (desc) so ELL segment-sum columns are prefixes.
- prop_L via: AllGather of dinv-scaled features (node-major table in DRAM),
  then ELL-structured indirect gather-accumulate DMAs (CCE add) into SBUF
  accumulators; self-loops folded into a per-node diagonal.
- All dense per-node math is feature-major [H=128 partitions, nodes free].
- Global layernorm = per-core partial sums + 8-core AllReduce of 2 scalars,
  applied as affine a*t+b.
- log_softmax in node-major after a PE transpose.
"""
import sys, os
sys.path.insert(0, '/opt/trn_rl_repo')
import numpy as np

from concourse import bass, mybir, tile, bacc, bass_utils

F32 = mybir.dt.float32
I32 = mybir.dt.int32
AF = mybir.ActivationFunctionType
OP = mybir.AluOpType

NCORES = 8
H = 128
KP1 = 3
NH = 4
HD = H // NH
PADROWS = 16
EPS = 1e-5
LRELU_ALPHA = 0.01


# ----------------------------------------------------------------- host side

def _preprocess(edge_index, N):
    row = edge_index[0].astype(np.int64)
    col = edge_index[1].astype(np.int64)
    E = row.shape[0]
    allrow = np.concatenate([row, np.arange(N)])
    allcol = np.concatenate([col, np.arange(N)])
    ones = np.ones(E + N, np.float64)
    deg = np.bincount(allcol, weights=ones, minlength=N)
    dinv = 1.0 / np.sqrt(np.maximum(deg, 1e-12))
    wn = dinv[allrow] * dinv[allcol]
    ldeg = np.bincount(allrow, weights=wn, minlength=N)
    diag = ldeg - dinv * dinv

    S = N // NCORES
    SP = S + PADROWS
    owner = col // S
    nonself_deg = np.bincount(col, minlength=N).astype(np.int64)
    perms, inv_perm = [], np.empty(N, np.int64)
    for c in range(NCORES):
        ids = np.arange(c * S, (c + 1) * S)
        order = np.argsort(-nonself_deg[ids], kind='stable')
        p = ids[order]
        perms.append(p)
        inv_perm[p] = c * S + np.arange(S)
    table_row = (inv_perm // S) * SP + (inv_perm % S)
    zrow_of_core = [c * SP + S for c in range(NCORES)]

    core_cols = []
    maxd = 0
    for c in range(NCORES):
        mask = owner == c
        er, ec = row[mask], col[mask]
        slot = inv_perm[ec] - c * S
        o = np.argsort(slot, kind='stable')
        er, slot = er[o], slot[o]
        degs = np.bincount(slot, minlength=S)
        jidx = np.arange(len(slot)) - np.concatenate([[0], np.cumsum(degs)])[slot]
        cols = []
        for j in range(int(degs.max()) if len(slot) else 0):
            m = jidx == j
            cols.append(table_row[er[m]].astype(np.int64))
        core_cols.append(cols)
        maxd = max(maxd, len(cols))

    CH = (S + 127) // 128
    sched = []
    for j in range(maxd):
        mc = 0
        for c in range(NCORES):
            if j < len(core_cols[c]):
                mc = max(mc, (len(core_cols[c][j]) + 127) // 128)
        for ch in range(mc):
            sched.append((j, ch))
    NCALL = len(sched)

    offs = []
    for c in range(NCORES):
        a = np.full((128, NCALL), zrow_of_core[c], np.int32)
        for k, (j, ch) in enumerate(sched):
            if j < len(core_cols[c]):
                colv = core_cols[c][j]
                lo, hi = ch * 128, min((ch + 1) * 128, len(colv))
                if hi > lo:
                    a[0:hi - lo, k] = colv[lo:hi]
        offs.append(a)

    pvecs = []
    for c in range(NCORES):
        dv = dinv[perms[c]].astype(np.float32)
        dg = diag[perms[c]].astype(np.float32)
        a = np.zeros((128, 2 * CH), np.float32)
        for ch in range(CH):
            lo, hi = ch * 128, min((ch + 1) * 128, S)
            a[0:hi - lo, ch] = dv[lo:hi]
            a[0:hi - lo, CH + ch] = dg[lo:hi]
        pvecs.append(a)

    return dict(perms=perms, offs=offs, pvecs=pvecs, sched=sched,
                NCALL=NCALL, CH=CH, S=S, SP=SP)


class _WPack:
    def __init__(self):
        self.cols, self.pos, self.n = [], {}, 0

    def add(self, name, arr):
        arr = np.asarray(arr, np.float32)
        assert arr.ndim == 2 and arr.shape[0] <= 128
        a = np.zeros((128, arr.shape[1]), np.float32)
        a[:arr.shape[0]] = arr
        self.pos[name] = (self.n, arr.shape[1])
        self.cols.append(a)
        self.n += arr.shape[1]

    def tensor(self):
        return np.concatenate(self.cols, axis=1)


def _pack_weights(params, NL):
    w = _WPack()
    w.add('eye', np.eye(128, dtype=np.float32))
    w.add('zero1', np.zeros((128, 1), np.float32))
    w.add('eps1', np.full((128, 1), EPS, np.float32))
    w.add('ones128', np.ones((128, 1), np.float32))
    w.add('ones1x', np.ones((1, 128), np.float32))
    S4 = np.zeros((128, NH), np.float32)
    for d in range(H):
        S4[d, d // HD] = 1.0
    w.add('S4', S4)
    B4 = np.zeros((64 + NH, 128), np.float32)
    for d in range(H):
        for p0 in (0, 32, 64):
            B4[p0 + d // HD, d] = 1.0
    w.add('B4', B4)
    encW = np.asarray(params['enc_W'], np.float32)
    F_IN = encW.shape[0]
    FP = ((F_IN + 127) // 128) * 128
    encWp = np.zeros((FP, H), np.float32)
    encWp[:F_IN] = encW
    for kx in range(FP // 128):
        w.add(f'encW{kx}', encWp[kx * 128:(kx + 1) * 128])
    w.add('encb', np.asarray(params['enc_b'], np.float32)[:, None])
    w.add('decW', np.asarray(params['dec_W'], np.float32))
    w.add('decb', np.asarray(params['dec_b'], np.float32)[:, None])
    # per-layer blocks, identical layout
    lpacks = []
    for l in range(NL):
        p = params['layers'][l]
        lw = _WPack()
        for k in range(KP1):
            lw.add(f'W1_{k}', np.asarray(p['linW1'][k], np.float32))
            lw.add(f'b1_{k}', np.asarray(p['linb1'][k], np.float32)[:, None])
            lw.add(f'W2_{k}', np.asarray(p['linW2'][k], np.float32))
            lw.add(f'b2_{k}', np.asarray(p['linb2'][k], np.float32)[:, None])
        lw.add('Wq', np.asarray(p['Wq'], np.float32))
        lw.add('bq', np.asarray(p['bq'], np.float32)[:, None])
        lw.add('Wk', np.asarray(p['Wk'], np.float32))
        lw.add('bk', np.asarray(p['bk'], np.float32)[:, None])
        f1W = np.asarray(p['f1W'], np.float32)
        for m in range(4):
            lw.add(f'f1W{m}', f1W[:, m * H:(m + 1) * H])
            lw.add(f'f1b{m}', np.asarray(p['f1b'], np.float32)[m * H:(m + 1) * H][:, None])
        f2W = np.asarray(p['f2W'], np.float32)
        for m in range(4):
            lw.add(f'f2W{m}', f2W[m * H:(m + 1) * H])
        lw.add('f2b3', 3.0 * np.asarray(p['f2b'], np.float32)[:, None])
        bias = (np.asarray(p['B'], np.float32) * np.asarray(p['hb'], np.float32))
        hv = np.zeros((H, KP1), np.float32)
        for d in range(H):
            hv[d] = bias[d // HD]
        for k in range(KP1):
            lw.add(f'hv{k}', hv[:, k][:, None])
        lpacks.append(lw)
    return w, lpacks, FP


# --------------------------------------------------------------- device side

def _build(cfg):
    N, NL, Cc = cfg['N'], cfg['NL'], cfg['C']
    S, SP, CH, NCALL = cfg['S'], cfg['SP'], cfg['CH'], cfg['NCALL']
    FP, NW = cfg['FP'], cfg['NW']
    NWL = cfg['NWL']
    sched = cfg['sched']
    wpos = cfg['wpos']
    lwpos = cfg['lwpos']
    TBL = SP * NCORES
    count1 = float(N * KP1 * H)
    count3 = float(N * H)
    NACC = 4                              # parallel accumulator strips

    nc = bacc.Bacc("TRN2", target_bir_lowering=False, debug=False,
                   enable_asserts=True, num_devices=NCORES)
    xT_t = nc.dram_tensor("xT", [FP, S], F32, kind="ExternalInput")
    offs_t = nc.dram_tensor("offs", [128, NCALL], I32, kind="ExternalInput")
    pvec_t = nc.dram_tensor("pvec", [128, 2 * CH], F32, kind="ExternalInput")
    wts_t = nc.dram_tensor("wts", [128, NW], F32, kind="ExternalInput")
    lwts_t = nc.dram_tensor("lwts", [NL, 128, NWL], F32, kind="ExternalInput")
    out_t = nc.dram_tensor("out", [S, Cc], F32, kind="ExternalOutput")

    ACC_CH = (CH + NACC - 1) // NACC      # chunks per strip

    with tile.TileContext(nc) as tc:
        with tc.tile_pool(name="persist", bufs=1) as pp, \
             tc.tile_pool(name="work", bufs=1) as wp, \
             tc.tile_pool(name="psA", bufs=2, space="PSUM") as psA, \
             tc.tile_pool(name="psB", bufs=1, space="PSUM") as psB, \
             tc.tile_pool(name="dram", bufs=2, space="DRAM") as dp, \
             tc.tile_pool(name="dram1", bufs=2, space="DRAM") as dp1:

            wts = pp.tile([128, NW], F32)
            nc.sync.dma_start(wts[:], wts_t.ap())
            offs = pp.tile([128, NCALL], I32)
            nc.sync.dma_start(offs[:], offs_t.ap())
            pvec = pp.tile([128, 2 * CH], F32)
            nc.sync.dma_start(pvec[:], pvec_t.ap())

            cur_lw = [None]

            def W(name):
                if name in wpos:
                    o, n = wpos[name]
                    return wts[:, o:o + n]
                o, n = lwpos[name]
                return cur_lw[0][:, o:o + n]

            def Wr(name, rows):
                if name in wpos:
                    o, n = wpos[name]
                    return wts[0:rows, o:o + n]
                o, n = lwpos[name]
                return cur_lw[0][0:rows, o:o + n]

            eye = W('eye')
            t0f = pp.tile([128, S], F32)
            t1f = pp.tile([128, S], F32)
            t2f = pp.tile([128, S], F32)
            tnB = pp.tile([128, CH * H], F32)
            accs = [pp.tile([128, ACC_CH * H], F32, name=f"acc{i}") for i in range(NACC)]
            zsb = pp.tile([PADROWS, H], F32)
            nc.vector.memset(zsb[:], 0.0)
            lnsb = pp.tile([1, 8], F32)
            lnsb2 = pp.tile([1, 2], F32)
            ab128 = pp.tile([128, 2], F32)
            part = pp.tile([128, 8], F32)
            sqacc = pp.tile([128, 1], F32)

            def acc_ap(ch):
                i, q = ch % NACC, ch // NACC
                return accs[i][:, q * H:(q + 1) * H]

            def nw_of(ch):
                return min(128, S - ch * 128)

            CS = []
            s = 0
            while s < S:
                CS.append((s, min(512, S - s)))
                s += 512

            # ---------------- encoder (streamed) ----------------
            with tc.tile_pool(name="xp", bufs=2) as xp:
                for s0, cw in CS:
                    ps = psA.tile([128, 512], F32, tag="mm")
                    for kx in range(FP // 128):
                        xt = xp.tile([128, 512], F32, tag="xt")
                        nc.sync.dma_start(xt[:, :cw],
                                          xT_t.ap()[kx * 128:(kx + 1) * 128, s0:s0 + cw])
                        nc.tensor.matmul(out=ps[:, :cw], lhsT=W(f'encW{kx}'),
                                         rhs=xt[:, :cw],
                                         start=(kx == 0), stop=(kx == FP // 128 - 1))
                    nc.scalar.activation(out=t0f[:, s0:s0 + cw], in_=ps[:, :cw],
                                         func=AF.Identity, bias=W('encb'))

            # ---------------- helpers ----------------
            def ln_allreduce(tiles, count):
                nhop = len(tiles)
                for kk, t in enumerate(tiles):
                    nc.vector.tensor_reduce(out=part[:, kk:kk + 1], in_=t[:, :S],
                                            axis=mybir.AxisListType.X, op=OP.add)
                    for ci, (s0, cw) in enumerate(CS):
                        scr = wp.tile([128, 512], F32, tag="scr", bufs=2)
                        nc.scalar.activation(
                            out=scr[:, :cw], in_=t[:, s0:s0 + cw], func=AF.Square,
                            bias=W('zero1'), accum_out=sqacc[:])
                        dst = part[:, nhop + kk:nhop + kk + 1]
                        if ci == 0:
                            nc.vector.tensor_copy(dst, sqacc[:])
                        else:
                            nc.vector.tensor_add(dst, dst, sqacc[:])
                ps = psB.tile([1, 8], F32, tag="lnx")
                nc.tensor.matmul(out=ps[:, :2 * nhop], lhsT=W('ones128'),
                                 rhs=part[:, :2 * nhop], start=True, stop=True)
                nc.vector.tensor_reduce(out=lnsb[:, 0:1], in_=ps[0:1, 0:nhop],
                                        axis=mybir.AxisListType.X, op=OP.add)
                nc.vector.tensor_reduce(out=lnsb[:, 1:2], in_=ps[0:1, nhop:2 * nhop],
                                        axis=mybir.AxisListType.X, op=OP.add)
                lin = dp1.tile([1, 8], F32, tag="lnin")
                lout = dp1.tile([1, 8], F32, tag="lnout")
                nc.sync.dma_start(lin[0:1, :], zsb[0:1, 0:8])
                nc.sync.dma_start(lin[0:1, 0:2], lnsb[:, 0:2])
                nc.gpsimd.collective_compute(
                    "AllReduce", OP.add, replica_groups=[list(range(NCORES))],
                    ins=[lin.opt()], outs=[lout.opt()])
                nc.sync.dma_start(lnsb[:, 0:2], lout[0:1, 0:2])
                nc.vector.tensor_scalar_mul(lnsb[:, 2:3], lnsb[:, 0:1], 1.0 / count)
                nc.vector.tensor_scalar_mul(lnsb[:, 3:4], lnsb[:, 1:2], 1.0 / count)
                nc.vector.tensor_mul(lnsb[:, 4:5], lnsb[:, 2:3], lnsb[:, 2:3])
                nc.vector.tensor_sub(lnsb[:, 5:6], lnsb[:, 3:4], lnsb[:, 4:5])
                nc.scalar.activation(out=lnsb[:, 6:7], in_=lnsb[:, 5:6],
                                     func=AF.Sqrt, bias=Wr('eps1', 1))
                nc.vector.reciprocal(lnsb2[:, 0:1], lnsb[:, 6:7])
                nc.vector.tensor_mul(lnsb2[:, 1:2], lnsb2[:, 0:1], lnsb[:, 2:3])
                nc.vector.tensor_scalar_mul(lnsb2[:, 1:2], lnsb2[:, 1:2], -1.0)
                bc = psB.tile([128, 2], F32, tag="lnx")
                nc.tensor.matmul(out=bc[:], lhsT=Wr('ones1x', 1), rhs=lnsb2[:],
                                 start=True, stop=True)
                nc.vector.tensor_copy(ab128[:], bc[:])

            def prop(src_feat, out_feat, first):
                """tnB <- diag*src - dinv*segsum(dinv*src); out_feat <- transpose."""
                ag = dp.tile([SP, H], F32, tag="agin")
                tbl = dp.tile([TBL, H], F32, tag="table")
                for ch in range(CH):
                    nw = nw_of(ch)
                    dv = pvec[0:nw, ch:ch + 1]
                    dg = pvec[0:nw, CH + ch:CH + ch + 1]
                    gst = wp.tile([128, H], F32, tag="gst", bufs=3)
                    if first:
                        pst = psA.tile([128, 128], F32, tag="tp")
                        nc.tensor.transpose(out=pst[0:nw, :],
                                            in_=src_feat[:, ch * 128:ch * 128 + nw],
                                            identity=eye)
                        nc.vector.tensor_scalar_mul(gst[0:nw, :], pst[0:nw, :], dv)
                        nc.vector.tensor_scalar_mul(tnB[0:nw, ch * H:(ch + 1) * H],
                                                    pst[0:nw, :], dg)
                    else:
                        tb = tnB[0:nw, ch * H:(ch + 1) * H]
                        nc.vector.tensor_scalar_mul(gst[0:nw, :], tb, dv)
                        nc.vector.tensor_scalar_mul(tb, tb, dg)
                    nc.sync.dma_start(ag[ch * 128:ch * 128 + nw, :], gst[0:nw, :])
                nc.sync.dma_start(ag[S:SP, :], zsb[:])
                nc.gpsimd.collective_compute(
                    "AllGather", OP.bypass, replica_groups=[list(range(NCORES))],
                    ins=[ag.opt()], outs=[tbl.opt()])
                for i in range(NACC):
                    nc.vector.memset(accs[i][:], 0.0)
                for k, (j, ch) in enumerate(sched):
                    nc.gpsimd.indirect_dma_start(
                        out=acc_ap(ch), out_offset=None, in_=tbl[:],
                        in_offset=bass.IndirectOffsetOnAxis(ap=offs[:, k:k + 1], axis=0),
                        compute_op=OP.add)
                for ch in range(CH):
                    nw = nw_of(ch)
                    dv = pvec[0:nw, ch:ch + 1]
                    tb = tnB[0:nw, ch * H:(ch + 1) * H]
                    tmp = wp.tile([128, H], F32, tag="ptmp", bufs=3)
                    nc.vector.tensor_scalar_mul(tmp[0:nw, :], acc_ap(ch)[0:nw, :], dv)
                    nc.vector.tensor_sub(tb, tb, tmp[0:nw, :])
                    pst = psA.tile([128, 128], F32, tag="tp")
                    nc.tensor.transpose(out=pst[:, 0:nw], in_=tb,
                                        identity=eye[0:nw, 0:nw])
                    nc.scalar.activation(out=out_feat[:, ch * 128:ch * 128 + nw],
                                         in_=pst[:, 0:nw], func=AF.Copy)

            # ---------------- layers ----------------
            hf = t0f
            inv_sqrt_hd = 1.0 / float(np.sqrt(HD))
            with tc.tile_pool(name="lwp", bufs=2) as lwp:
              for l in range(NL):
                lw = lwp.tile([128, NWL], F32, tag="lw")
                nc.sync.dma_start(lw[:], lwts_t.ap()[l, :, :])
                cur_lw[0] = lw
                prop(hf, t1f, first=True)
                prop(None, t2f, first=False)
                toks = [hf, t1f, t2f]
                ln_allreduce(toks, count1)
                for kk in range(KP1):
                    nc.vector.tensor_scalar(
                        out=toks[kk][:, :S], in0=toks[kk][:, :S],
                        scalar1=ab128[:, 0:1], scalar2=ab128[:, 1:2],
                        op0=OP.mult, op1=OP.add)
                for kk in range(KP1):
                    for s0, cw in CS:
                        ps1 = psA.tile([128, 512], F32, tag="mm")
                        nc.tensor.matmul(out=ps1[:, :cw], lhsT=W(f'W1_{kk}'),
                                         rhs=toks[kk][:, s0:s0 + cw], start=True, stop=True)
                        m1 = wp.tile([128, 512], F32, tag="m1", bufs=2)
                        nc.scalar.activation(out=m1[:, :cw], in_=ps1[:, :cw],
                                             func=AF.Lrelu, bias=W(f'b1_{kk}'),
                                             alpha=LRELU_ALPHA)
                        ps2 = psA.tile([128, 512], F32, tag="mm")
                        nc.tensor.matmul(out=ps2[:, :cw], lhsT=W(f'W2_{kk}'),
                                         rhs=m1[:, :cw], start=True, stop=True)
                        nc.scalar.activation(out=toks[kk][:, s0:s0 + cw], in_=ps2[:, :cw],
                                             func=AF.Identity, bias=W(f'b2_{kk}'))
                # attention
                for s0, cw in CS:
                    qkv = wp.tile([128, 9 * 512], F32, tag="qkv")
                    for kk in range(KP1):
                        psq = psA.tile([128, 512], F32, tag="mm")
                        nc.tensor.matmul(out=psq[:, :cw], lhsT=W('Wq'),
                                         rhs=toks[kk][:, s0:s0 + cw], start=True, stop=True)
                        nc.scalar.activation(out=qkv[:, kk * 512:kk * 512 + cw],
                                             in_=psq[:, :cw], func=AF.Identity,
                                             bias=W('bq'))
                        psk = psA.tile([128, 512], F32, tag="mm")
                        nc.tensor.matmul(out=psk[:, :cw], lhsT=W('Wk'),
                                         rhs=toks[kk][:, s0:s0 + cw], start=True, stop=True)
                        nc.scalar.activation(out=qkv[:, (3 + kk) * 512:(3 + kk) * 512 + cw],
                                             in_=psk[:, :cw], func=AF.Identity,
                                             bias=W('bk'))
                        nc.vector.tensor_scalar_mul(qkv[:, (6 + kk) * 512:(6 + kk) * 512 + cw],
                                                    toks[kk][:, s0:s0 + cw],
                                                    W(f'hv{kk}'))
                    scsb = wp.tile([64 + NH, KP1 * 512], F32, tag="scsb")
                    tab = wp.tile([128, 512], F32, tag="tab", bufs=2)
                    for a in range(KP1):
                        for b in range(KP1):
                            nc.vector.tensor_mul(tab[:, :cw], qkv[:, a * 512:a * 512 + cw],
                                                 qkv[:, (3 + b) * 512:(3 + b) * 512 + cw])
                            pr = a * KP1 + b
                            p0, c0 = 32 * (pr % 3), (pr // 3) * 512
                            scps = psB.tile([NH, 512], F32, tag="sc")
                            nc.tensor.matmul(out=scps[:, :cw], lhsT=W('S4'),
                                             rhs=tab[:, :cw], start=True, stop=True)
                            nc.scalar.activation(out=scsb[p0:p0 + NH, c0:c0 + cw],
                                                 in_=scps[:, :cw],
                                                 func=AF.Tanh, bias=Wr('zero1', NH),
                                                 scale=inv_sqrt_hd)
                    for a in range(KP1):
                        attn = wp.tile([128, 512], F32, tag="attn")
                        for b in range(KP1):
                            pr = a * KP1 + b
                            p0, c0 = 32 * (pr % 3), (pr // 3) * 512
                            bc = psB.tile([128, 512], F32, tag="bc")
                            b4o, b4n = (wpos['B4'][0], wpos['B4'][1])
                            nc.tensor.matmul(out=bc[:, :cw],
                                             lhsT=wts[p0:p0 + NH, b4o:b4o + b4n],
                                             rhs=scsb[p0:p0 + NH, c0:c0 + cw], start=True, stop=True)
                            vb = qkv[:, (6 + b) * 512:(6 + b) * 512 + cw]
                            if b == 0:
                                nc.vector.tensor_mul(attn[:, :cw], bc[:, :cw], vb)
                            else:
                                tmp = wp.tile([128, 512], F32, tag="atmp", bufs=2)
                                nc.vector.tensor_mul(tmp[:, :cw], bc[:, :cw], vb)
                                nc.vector.tensor_add(attn[:, :cw], attn[:, :cw], tmp[:, :cw])
                        nc.vector.tensor_add(toks[a][:, s0:s0 + cw],
                                             toks[a][:, s0:s0 + cw], attn[:, :cw])
                # LN2 + FFN (writes hf in place)
                ln_allreduce(toks, count1)
                for s0, cw in CS:
                    psh = psB.tile([128, 512], F32, tag="bc")
                    first_mm = True
                    for a in range(KP1):
                        xa = wp.tile([128, 512], F32, tag="xa", bufs=2)
                        nc.vector.tensor_scalar(
                            out=xa[:, :cw], in0=toks[a][:, s0:s0 + cw],
                            scalar1=ab128[:, 0:1], scalar2=ab128[:, 1:2],
                            op0=OP.mult, op1=OP.add)
                        for m in range(4):
                            psf = psA.tile([128, 512], F32, tag="mm")
                            nc.tensor.matmul(out=psf[:, :cw], lhsT=W(f'f1W{m}'),
                                             rhs=xa[:, :cw], start=True, stop=True)
                            rl = wp.tile([128, 512], F32, tag="rl", bufs=2)
                            nc.scalar.activation(out=rl[:, :cw], in_=psf[:, :cw],
                                                 func=AF.Lrelu, bias=W(f'f1b{m}'),
                                                 alpha=LRELU_ALPHA)
                            nc.tensor.matmul(out=psh[:, :cw], lhsT=W(f'f2W{m}'),
                                             rhs=rl[:, :cw], start=first_mm,
                                             stop=(a == KP1 - 1 and m == 3))
                            first_mm = False
                    nc.scalar.activation(out=hf[:, s0:s0 + cw], in_=psh[:, :cw],
                                         func=AF.Identity, bias=W('f2b3'))

            # ---------------- final LN + decoder + log_softmax ----------------
            ln_allreduce([hf], count3)
            nc.vector.tensor_scalar(out=hf[:, :S], in0=hf[:, :S],
                                    scalar1=ab128[:, 0:1], scalar2=ab128[:, 1:2],
                                    op0=OP.mult, op1=OP.add)
            for s0, cw in CS:
                psd = psA.tile([128, 512], F32, tag="mm")
                nc.tensor.matmul(out=psd[0:Cc, :cw], lhsT=W('decW'),
                                 rhs=hf[:, s0:s0 + cw], start=True, stop=True)
                dsb = wp.tile([Cc, 512], F32, tag="dsb", bufs=2)
                nc.scalar.activation(out=dsb[:, :cw], in_=psd[0:Cc, :cw],
                                     func=AF.Identity, bias=Wr('decb', Cc))
                b0 = 0
                while b0 < cw:
                    bw = min(128, cw - b0)
                    pst = psA.tile([128, 128], F32, tag="tp")
                    nc.tensor.transpose(out=pst[0:bw, 0:Cc], in_=dsb[:, b0:b0 + bw],
                                        identity=eye[0:Cc, 0:Cc])
                    lg = wp.tile([128, Cc], F32, tag="lg", bufs=2)
                    nc.scalar.activation(out=lg[0:bw, :], in_=pst[0:bw, 0:Cc], func=AF.Copy)
                    mx = wp.tile([128, 2], F32, tag="mx", bufs=2)
                    nc.vector.tensor_reduce(out=mx[0:bw, 0:1], in_=lg[0:bw, :],
                                            axis=mybir.AxisListType.X, op=OP.max,
                                            negate=True)
                    ex = wp.tile([128, Cc], F32, tag="ex", bufs=2)
                    sume = wp.tile([128, 2], F32, tag="sume", bufs=2)
                    nc.scalar.activation(out=ex[0:bw, :], in_=lg[0:bw, :], func=AF.Exp,
                                         bias=mx[0:bw, 0:1], accum_out=sume[0:bw, 0:1])
                    lnz = wp.tile([128, 2], F32, tag="lnz", bufs=2)
                    nc.scalar.activation(out=lnz[0:bw, 0:1], in_=sume[0:bw, 0:1],
                                         func=AF.Ln, bias=Wr('zero1', bw))
                    fin = wp.tile([128, Cc], F32, tag="fin", bufs=2)
                    nc.vector.tensor_scalar(
                        out=fin[0:bw, :], in0=lg[0:bw, :],
                        scalar1=mx[0:bw, 0:1], scalar2=lnz[0:bw, 0:1],
                        op0=OP.add, op1=OP.subtract)
                    nc.sync.dma_start(out_t.ap()[s0 + b0:s0 + b0 + bw, :], fin[0:bw, :])
                    b0 += bw

    nc.compile()
    return nc


# ------------------------------------------------------------------- driver

_CACHE = {}


def run(x, edge_index, params, NL):
    x = np.asarray(x, np.float32)
    edge_index = np.asarray(edge_index)
    N = x.shape[0]
    pre = _preprocess(edge_index, N)
    w, lpacks, FP = _pack_weights(params, NL)
    wts_np = w.tensor()
    lwts_np = np.stack([lp.tensor() for lp in lpacks], axis=0)
    C = np.asarray(params['dec_W']).shape[1]
    cfg = dict(N=N, NL=NL, C=C, S=pre['S'], SP=pre['SP'], CH=pre['CH'],
               NCALL=pre['NCALL'], FP=FP, NW=wts_np.shape[1],
               NWL=lwts_np.shape[2], wpos=w.pos, lwpos=lpacks[0].pos,
               sched=pre['sched'])
    key = (N, NL, C, pre['NCALL'], FP, wts_np.shape[1], lwts_np.shape[2],
           tuple(pre['sched'][:8]))
    if key not in _CACHE:
        _CACHE[key] = _build(cfg)
    nc = _CACHE[key]

    S = pre['S']
    in_maps = []
    for c in range(NCORES):
        xp = np.zeros((FP, S), np.float32)
        xp[:x.shape[1], :] = x[pre['perms'][c]].T
        in_maps.append({"xT": xp, "offs": pre['offs'][c],
                        "pvec": pre['pvecs'][c], "wts": wts_np,
                        "lwts": lwts_np})
    res = bass_utils.run_bass_kernel_spmd(nc, in_maps, core_ids=list(range(NCORES)),
                                          trace=os.environ.get('KTRACE', '0') == '1')
    out = np.empty((N, C), np.float32)
    for c in range(NCORES):
        out[pre['perms'][c]] = res.results[c]['out']
    run.last_exec_time_ns = res.exec_time_ns
    return out


run.last_exec_time_ns = None


def kernel(x, edge_index, params):
    return run(x, edge_index, params, NL=len(params['layers']))


# revision 13
# speedup vs baseline: 1.0287x; 1.0287x over previous
"""Distributed Trainium2 Bass kernel for nn_DiffusedAttention (GNN message passing).

Strategy:
- Nodes sharded 6250/core across 8 NeuronCores; per-core nodes permuted by
  in-degree (desc) so ELL segment-sum columns are prefixes.
- prop_L via: AllGather of dinv-scaled features (node-major table in DRAM),
  then ELL-structured indirect gather-accumulate DMAs (CCE add) into SBUF
  accumulators; self-loops folded into a per-node diagonal.
- All dense per-node math is feature-major [H=128 partitions, nodes free].
- Global layernorm = per-core partial sums + 8-core AllReduce of 2 scalars,
  applied as affine a*t+b.
- log_softmax in node-major after a PE transpose.
"""
import sys, os
sys.path.insert(0, '/opt/trn_rl_repo')
import numpy as np

from concourse import bass, mybir, tile, bacc, bass_utils

F32 = mybir.dt.float32
I32 = mybir.dt.int32
AF = mybir.ActivationFunctionType
OP = mybir.AluOpType

NCORES = 8
H = 128
KP1 = 3
NH = 4
HD = H // NH
PADROWS = 16
EPS = 1e-5
LRELU_ALPHA = 0.01


# ----------------------------------------------------------------- host side

def _preprocess(edge_index, N):
    row = edge_index[0].astype(np.int64)
    col = edge_index[1].astype(np.int64)
    E = row.shape[0]
    allrow = np.concatenate([row, np.arange(N)])
    allcol = np.concatenate([col, np.arange(N)])
    ones = np.ones(E + N, np.float64)
    deg = np.bincount(allcol, weights=ones, minlength=N)
    dinv = 1.0 / np.sqrt(np.maximum(deg, 1e-12))
    wn = dinv[allrow] * dinv[allcol]
    ldeg = np.bincount(allrow, weights=wn, minlength=N)
    diag = ldeg - dinv * dinv

    S = N // NCORES
    SP = S + PADROWS
    owner = col // S
    nonself_deg = np.bincount(col, minlength=N).astype(np.int64)
    perms, inv_perm = [], np.empty(N, np.int64)
    for c in range(NCORES):
        ids = np.arange(c * S, (c + 1) * S)
        order = np.argsort(-nonself_deg[ids], kind='stable')
        p = ids[order]
        perms.append(p)
        inv_perm[p] = c * S + np.arange(S)
    table_row = (inv_perm // S) * SP + (inv_perm % S)
    zrow_of_core = [c * SP + S for c in range(NCORES)]

    core_cols = []
    maxd = 0
    for c in range(NCORES):
        mask = owner == c
        er, ec = row[mask], col[mask]
        slot = inv_perm[ec] - c * S
        o = np.argsort(slot, kind='stable')
        er, slot = er[o], slot[o]
        degs = np.bincount(slot, minlength=S)
        jidx = np.arange(len(slot)) - np.concatenate([[0], np.cumsum(degs)])[slot]
        cols = []
        for j in range(int(degs.max()) if len(slot) else 0):
            m = jidx == j
            cols.append(table_row[er[m]].astype(np.int64))
        core_cols.append(cols)
        maxd = max(maxd, len(cols))

    CH = (S + 127) // 128
    sched = []
    for j in range(maxd):
        mc = 0
        for c in range(NCORES):
            if j < len(core_cols[c]):
                mc = max(mc, (len(core_cols[c][j]) + 127) // 128)
        for ch in range(mc):
            sched.append((j, ch))
    NCALL = len(sched)

    offs = []
    for c in range(NCORES):
        a = np.full((128, NCALL), zrow_of_core[c], np.int32)
        for k, (j, ch) in enumerate(sched):
            if j < len(core_cols[c]):
                colv = core_cols[c][j]
                lo, hi = ch * 128, min((ch + 1) * 128, len(colv))
                if hi > lo:
                    a[0:hi - lo, k] = colv[lo:hi]
        offs.append(a)

    pvecs = []
    for c in range(NCORES):
        dv = dinv[perms[c]].astype(np.float32)
        dg = diag[perms[c]].astype(np.float32)
        a = np.zeros((128, 2 * CH), np.float32)
        for ch in range(CH):
            lo, hi = ch * 128, min((ch + 1) * 128, S)
            a[0:hi - lo, ch] = dv[lo:hi]
            a[0:hi - lo, CH + ch] = dg[lo:hi]
        pvecs.append(a)

    return dict(perms=perms, offs=offs, pvecs=pvecs, sched=sched,
                NCALL=NCALL, CH=CH, S=S, SP=SP)


class _WPack:
    def __init__(self):
        self.cols, self.pos, self.n = [], {}, 0

    def add(self, name, arr):
        arr = np.asarray(arr, np.float32)
        assert arr.ndim == 2 and arr.shape[0] <= 128
        a = np.zeros((128, arr.shape[1]), np.float32)
        a[:arr.shape[0]] = arr
        self.pos[name] = (self.n, arr.shape[1])
        self.cols.append(a)
        self.n += arr.shape[1]

    def tensor(self):
        return np.concatenate(self.cols, axis=1)


def _pack_weights(params, NL):
    w = _WPack()
    w.add('eye', np.eye(128, dtype=np.float32))
    w.add('zero1', np.zeros((128, 1), np.float32))
    w.add('eps1', np.full((128, 1), EPS, np.float32))
    w.add('ones128', np.ones((128, 1), np.float32))
    w.add('ones1x', np.ones((1, 128), np.float32))
    S4 = np.zeros((128, NH), np.float32)
    for d in range(H):
        S4[d, d // HD] = 1.0
    w.add('S4', S4)
    B4 = np.zeros((64 + NH, 128), np.float32)
    for d in range(H):
        for p0 in (0, 32, 64):
            B4[p0 + d // HD, d] = 1.0
    w.add('B4', B4)
    encW = np.asarray(params['enc_W'], np.float32)
    F_IN = encW.shape[0]
    FP = ((F_IN + 127) // 128) * 128
    encWp = np.zeros((FP, H), np.float32)
    encWp[:F_IN] = encW
    for kx in range(FP // 128):
        w.add(f'encW{kx}', encWp[kx * 128:(kx + 1) * 128])
    w.add('encb', np.asarray(params['enc_b'], np.float32)[:, None])
    w.add('decW', np.asarray(params['dec_W'], np.float32))
    w.add('decb', np.asarray(params['dec_b'], np.float32)[:, None])
    # per-layer blocks, identical layout
    lpacks = []
    for l in range(NL):
        p = params['layers'][l]
        lw = _WPack()
        for k in range(KP1):
            lw.add(f'W1_{k}', np.asarray(p['linW1'][k], np.float32))
            lw.add(f'b1_{k}', np.asarray(p['linb1'][k], np.float32)[:, None])
            lw.add(f'W2_{k}', np.asarray(p['linW2'][k], np.float32))
            lw.add(f'b2_{k}', np.asarray(p['linb2'][k], np.float32)[:, None])
        lw.add('Wq', np.asarray(p['Wq'], np.float32))
        lw.add('bq', np.asarray(p['bq'], np.float32)[:, None])
        lw.add('Wk', np.asarray(p['Wk'], np.float32))
        lw.add('bk', np.asarray(p['bk'], np.float32)[:, None])
        f1W = np.asarray(p['f1W'], np.float32)
        for m in range(4):
            lw.add(f'f1W{m}', f1W[:, m * H:(m + 1) * H])
            lw.add(f'f1b{m}', np.asarray(p['f1b'], np.float32)[m * H:(m + 1) * H][:, None])
        f2W = np.asarray(p['f2W'], np.float32)
        for m in range(4):
            lw.add(f'f2W{m}', f2W[m * H:(m + 1) * H])
        lw.add('f2b3', 3.0 * np.asarray(p['f2b'], np.float32)[:, None])
        bias = (np.asarray(p['B'], np.float32) * np.asarray(p['hb'], np.float32))
        hv = np.zeros((H, KP1), np.float32)
        for d in range(H):
            hv[d] = bias[d // HD]
        for k in range(KP1):
            lw.add(f'hv{k}', hv[:, k][:, None])
        lpacks.append(lw)
    return w, lpacks, FP


# --------------------------------------------------------------- device side

def _build(cfg):
    N, NL, Cc = cfg['N'], cfg['NL'], cfg['C']
    S, SP, CH, NCALL = cfg['S'], cfg['SP'], cfg['CH'], cfg['NCALL']
    FP, NW = cfg['FP'], cfg['NW']
    NWL = cfg['NWL']
    sched = cfg['sched']
    wpos = cfg['wpos']
    lwpos = cfg['lwpos']
    TBL = SP * NCORES
    count1 = float(N * KP1 * H)
    count3 = float(N * H)


    nc = bacc.Bacc("TRN2", target_bir_lowering=False, debug=False,
                   enable_asserts=True, num_devices=NCORES)
    xT_t = nc.dram_tensor("xT", [FP, S], F32, kind="ExternalInput")
    offs_t = nc.dram_tensor("offs", [128, NCALL], I32, kind="ExternalInput")
    pvec_t = nc.dram_tensor("pvec", [128, 2 * CH], F32, kind="ExternalInput")
    wts_t = nc.dram_tensor("wts", [128, NW], F32, kind="ExternalInput")
    lwts_t = nc.dram_tensor("lwts", [NL, 128, NWL], F32, kind="ExternalInput")
    out_t = nc.dram_tensor("out", [S, Cc], F32, kind="ExternalOutput")

    with tile.TileContext(nc) as tc:
        with tc.tile_pool(name="persist", bufs=1) as pp, \
             tc.tile_pool(name="work", bufs=1) as wp, \
             tc.tile_pool(name="psA", bufs=2, space="PSUM") as psA, \
             tc.tile_pool(name="psB", bufs=1, space="PSUM") as psB, \
             tc.tile_pool(name="dram", bufs=2, space="DRAM") as dp, \
             tc.tile_pool(name="dram1", bufs=2, space="DRAM") as dp1:

            wts = pp.tile([128, NW], F32)
            nc.sync.dma_start(wts[:], wts_t.ap())
            offs = pp.tile([128, NCALL], I32)
            nc.sync.dma_start(offs[:], offs_t.ap())
            pvec = pp.tile([128, 2 * CH], F32)
            nc.sync.dma_start(pvec[:], pvec_t.ap())

            cur_lw = [None]

            def W(name):
                if name in wpos:
                    o, n = wpos[name]
                    return wts[:, o:o + n]
                o, n = lwpos[name]
                return cur_lw[0][:, o:o + n]

            def Wr(name, rows):
                if name in wpos:
                    o, n = wpos[name]
                    return wts[0:rows, o:o + n]
                o, n = lwpos[name]
                return cur_lw[0][0:rows, o:o + n]

            eye = W('eye')
            t0f = pp.tile([128, S], F32)
            t1f = pp.tile([128, S], F32)
            t2f = pp.tile([128, S], F32)
            tnB = pp.tile([128, CH * H], F32)
            accs = [pp.tile([128, H], F32, name=f"acc{i}") for i in range(CH)]
            zsb = pp.tile([PADROWS, H], F32)
            nc.vector.memset(zsb[:], 0.0)
            lnsb = pp.tile([1, 8], F32)
            lnsb2 = pp.tile([1, 2], F32)
            ab128 = pp.tile([128, 2], F32)
            part = pp.tile([128, 8], F32)
            sqacc = pp.tile([128, 1], F32)

            def acc_ap(ch):
                return accs[ch][:]

            def nw_of(ch):
                return min(128, S - ch * 128)

            CS = []
            s = 0
            while s < S:
                CS.append((s, min(512, S - s)))
                s += 512

            # ---------------- encoder (streamed) ----------------
            with tc.tile_pool(name="xp", bufs=2) as xp:
                for s0, cw in CS:
                    ps = psA.tile([128, 512], F32, tag="mm")
                    for kx in range(FP // 128):
                        xt = xp.tile([128, 512], F32, tag="xt")
                        nc.sync.dma_start(xt[:, :cw],
                                          xT_t.ap()[kx * 128:(kx + 1) * 128, s0:s0 + cw])
                        nc.tensor.matmul(out=ps[:, :cw], lhsT=W(f'encW{kx}'),
                                         rhs=xt[:, :cw],
                                         start=(kx == 0), stop=(kx == FP // 128 - 1))
                    nc.scalar.activation(out=t0f[:, s0:s0 + cw], in_=ps[:, :cw],
                                         func=AF.Identity, bias=W('encb'))

            # ---------------- helpers ----------------
            def ln_allreduce(tiles, count):
                nhop = len(tiles)
                for kk, t in enumerate(tiles):
                    nc.vector.tensor_reduce(out=part[:, kk:kk + 1], in_=t[:, :S],
                                            axis=mybir.AxisListType.X, op=OP.add)
                    for ci, (s0, cw) in enumerate(CS):
                        scr = wp.tile([128, 512], F32, tag="scr", bufs=2)
                        nc.scalar.activation(
                            out=scr[:, :cw], in_=t[:, s0:s0 + cw], func=AF.Square,
                            bias=W('zero1'), accum_out=sqacc[:])
                        dst = part[:, nhop + kk:nhop + kk + 1]
                        if ci == 0:
                            nc.vector.tensor_copy(dst, sqacc[:])
                        else:
                            nc.vector.tensor_add(dst, dst, sqacc[:])
                ps = psB.tile([1, 8], F32, tag="lnx")
                nc.tensor.matmul(out=ps[:, :2 * nhop], lhsT=W('ones128'),
                                 rhs=part[:, :2 * nhop], start=True, stop=True)
                nc.vector.tensor_reduce(out=lnsb[:, 0:1], in_=ps[0:1, 0:nhop],
                                        axis=mybir.AxisListType.X, op=OP.add)
                nc.vector.tensor_reduce(out=lnsb[:, 1:2], in_=ps[0:1, nhop:2 * nhop],
                                        axis=mybir.AxisListType.X, op=OP.add)
                lin = dp1.tile([1, 8], F32, tag="lnin")
                lout = dp1.tile([1, 8], F32, tag="lnout", addr_space="Shared")
                nc.sync.dma_start(lin[0:1, :], zsb[0:1, 0:8])
                nc.sync.dma_start(lin[0:1, 0:2], lnsb[:, 0:2])
                nc.gpsimd.collective_compute(
                    "AllReduce", OP.add, replica_groups=[list(range(NCORES))],
                    ins=[lin.opt()], outs=[lout.opt()])
                nc.sync.dma_start(lnsb[:, 0:2], lout[0:1, 0:2])
                nc.vector.tensor_scalar_mul(lnsb[:, 2:3], lnsb[:, 0:1], 1.0 / count)
                nc.vector.tensor_scalar_mul(lnsb[:, 3:4], lnsb[:, 1:2], 1.0 / count)
                nc.vector.tensor_mul(lnsb[:, 4:5], lnsb[:, 2:3], lnsb[:, 2:3])
                nc.vector.tensor_sub(lnsb[:, 5:6], lnsb[:, 3:4], lnsb[:, 4:5])
                nc.scalar.activation(out=lnsb[:, 6:7], in_=lnsb[:, 5:6],
                                     func=AF.Sqrt, bias=Wr('eps1', 1))
                nc.vector.reciprocal(lnsb2[:, 0:1], lnsb[:, 6:7])
                nc.vector.tensor_mul(lnsb2[:, 1:2], lnsb2[:, 0:1], lnsb[:, 2:3])
                nc.vector.tensor_scalar_mul(lnsb2[:, 1:2], lnsb2[:, 1:2], -1.0)
                bc = psB.tile([128, 2], F32, tag="lnx")
                nc.tensor.matmul(out=bc[:], lhsT=Wr('ones1x', 1), rhs=lnsb2[:],
                                 start=True, stop=True)
                nc.vector.tensor_copy(ab128[:], bc[:])

            def prop(src_feat, out_feat, first):
                """tnB <- diag*src - dinv*segsum(dinv*src); out_feat <- transpose."""
                ag = dp.tile([SP, H], F32, tag="agin")
                tbl = dp.tile([TBL, H], F32, tag="table", addr_space="Shared")
                for ch in range(CH):
                    nw = nw_of(ch)
                    dv = pvec[0:nw, ch:ch + 1]
                    dg = pvec[0:nw, CH + ch:CH + ch + 1]
                    gst = wp.tile([128, H], F32, tag="gst", bufs=3)
                    if first:
                        pst = psA.tile([128, 128], F32, tag="tp")
                        nc.tensor.transpose(out=pst[0:nw, :],
                                            in_=src_feat[:, ch * 128:ch * 128 + nw],
                                            identity=eye)
                        nc.vector.tensor_scalar_mul(gst[0:nw, :], pst[0:nw, :], dv)
                        nc.vector.tensor_scalar_mul(tnB[0:nw, ch * H:(ch + 1) * H],
                                                    pst[0:nw, :], dg)
                    else:
                        tb = tnB[0:nw, ch * H:(ch + 1) * H]
                        nc.vector.tensor_scalar_mul(gst[0:nw, :], tb, dv)
                        nc.vector.tensor_scalar_mul(tb, tb, dg)
                    nc.sync.dma_start(ag[ch * 128:ch * 128 + nw, :], gst[0:nw, :])
                nc.sync.dma_start(ag[S:SP, :], zsb[:])
                nc.gpsimd.collective_compute(
                    "AllGather", OP.bypass, replica_groups=[list(range(NCORES))],
                    ins=[ag.opt()], outs=[tbl.opt()])
                for i in range(CH):
                    nc.vector.memset(accs[i][:], 0.0)
                for k, (j, ch) in enumerate(sched):
                    nc.gpsimd.indirect_dma_start(
                        out=acc_ap(ch), out_offset=None, in_=tbl[:],
                        in_offset=bass.IndirectOffsetOnAxis(ap=offs[:, k:k + 1], axis=0),
                        compute_op=OP.add)
                for ch in range(CH):
                    nw = nw_of(ch)
                    dv = pvec[0:nw, ch:ch + 1]
                    tb = tnB[0:nw, ch * H:(ch + 1) * H]
                    tmp = wp.tile([128, H], F32, tag="ptmp", bufs=3)
                    nc.vector.tensor_scalar_mul(tmp[0:nw, :], acc_ap(ch)[0:nw, :], dv)
                    nc.vector.tensor_sub(tb, tb, tmp[0:nw, :])
                    pst = psA.tile([128, 128], F32, tag="tp")
                    nc.tensor.transpose(out=pst[:, 0:nw], in_=tb,
                                        identity=eye[0:nw, 0:nw])
                    nc.scalar.activation(out=out_feat[:, ch * 128:ch * 128 + nw],
                                         in_=pst[:, 0:nw], func=AF.Copy)

            # ---------------- layers ----------------
            hf = t0f
            inv_sqrt_hd = 1.0 / float(np.sqrt(HD))
            with tc.tile_pool(name="lwp", bufs=2) as lwp:
              for l in range(NL):
                lw = lwp.tile([128, NWL], F32, tag="lw")
                nc.sync.dma_start(lw[:], lwts_t.ap()[l, :, :])
                cur_lw[0] = lw
                prop(hf, t1f, first=True)
                prop(None, t2f, first=False)
                toks = [hf, t1f, t2f]
                ln_allreduce(toks, count1)
                for kk in range(KP1):
                    nc.vector.tensor_scalar(
                        out=toks[kk][:, :S], in0=toks[kk][:, :S],
                        scalar1=ab128[:, 0:1], scalar2=ab128[:, 1:2],
                        op0=OP.mult, op1=OP.add)
                for kk in range(KP1):
                    for s0, cw in CS:
                        ps1 = psA.tile([128, 512], F32, tag="mm")
                        nc.tensor.matmul(out=ps1[:, :cw], lhsT=W(f'W1_{kk}'),
                                         rhs=toks[kk][:, s0:s0 + cw], start=True, stop=True)
                        m1 = wp.tile([128, 512], F32, tag="m1", bufs=2)
                        nc.scalar.activation(out=m1[:, :cw], in_=ps1[:, :cw],
                                             func=AF.Lrelu, bias=W(f'b1_{kk}'),
                                             alpha=LRELU_ALPHA)
                        ps2 = psA.tile([128, 512], F32, tag="mm")
                        nc.tensor.matmul(out=ps2[:, :cw], lhsT=W(f'W2_{kk}'),
                                         rhs=m1[:, :cw], start=True, stop=True)
                        nc.scalar.activation(out=toks[kk][:, s0:s0 + cw], in_=ps2[:, :cw],
                                             func=AF.Identity, bias=W(f'b2_{kk}'))
                # attention
                for s0, cw in CS:
                    qkv = wp.tile([128, 9 * 512], F32, tag="qkv")
                    for kk in range(KP1):
                        psq = psA.tile([128, 512], F32, tag="mm")
                        nc.tensor.matmul(out=psq[:, :cw], lhsT=W('Wq'),
                                         rhs=toks[kk][:, s0:s0 + cw], start=True, stop=True)
                        nc.scalar.activation(out=qkv[:, kk * 512:kk * 512 + cw],
                                             in_=psq[:, :cw], func=AF.Identity,
                                             bias=W('bq'))
                        psk = psA.tile([128, 512], F32, tag="mm")
                        nc.tensor.matmul(out=psk[:, :cw], lhsT=W('Wk'),
                                         rhs=toks[kk][:, s0:s0 + cw], start=True, stop=True)
                        nc.scalar.activation(out=qkv[:, (3 + kk) * 512:(3 + kk) * 512 + cw],
                                             in_=psk[:, :cw], func=AF.Identity,
                                             bias=W('bk'))
                        nc.vector.tensor_scalar_mul(qkv[:, (6 + kk) * 512:(6 + kk) * 512 + cw],
                                                    toks[kk][:, s0:s0 + cw],
                                                    W(f'hv{kk}'))
                    scsb = wp.tile([64 + NH, KP1 * 512], F32, tag="scsb")
                    tab = wp.tile([128, 512], F32, tag="tab", bufs=2)
                    for a in range(KP1):
                        for b in range(KP1):
                            nc.vector.tensor_mul(tab[:, :cw], qkv[:, a * 512:a * 512 + cw],
                                                 qkv[:, (3 + b) * 512:(3 + b) * 512 + cw])
                            pr = a * KP1 + b
                            p0, c0 = 32 * (pr % 3), (pr // 3) * 512
                            scps = psB.tile([NH, 512], F32, tag="sc")
                            nc.tensor.matmul(out=scps[:, :cw], lhsT=W('S4'),
                                             rhs=tab[:, :cw], start=True, stop=True)
                            nc.scalar.activation(out=scsb[p0:p0 + NH, c0:c0 + cw],
                                                 in_=scps[:, :cw],
                                                 func=AF.Tanh, bias=Wr('zero1', NH),
                                                 scale=inv_sqrt_hd)
                    for a in range(KP1):
                        attn = wp.tile([128, 512], F32, tag="attn")
                        for b in range(KP1):
                            pr = a * KP1 + b
                            p0, c0 = 32 * (pr % 3), (pr // 3) * 512
                            bc = psB.tile([128, 512], F32, tag="bc", bufs=2)
                            b4o, b4n = (wpos['B4'][0], wpos['B4'][1])
                            nc.tensor.matmul(out=bc[:, :cw],
                                             lhsT=wts[p0:p0 + NH, b4o:b4o + b4n],
                                             rhs=scsb[p0:p0 + NH, c0:c0 + cw], start=True, stop=True)
                            vb = qkv[:, (6 + b) * 512:(6 + b) * 512 + cw]
                            if b == 0:
                                nc.vector.tensor_mul(attn[:, :cw], bc[:, :cw], vb)
                            else:
                                tmp = wp.tile([128, 512], F32, tag="atmp", bufs=2)
                                nc.vector.tensor_mul(tmp[:, :cw], bc[:, :cw], vb)
                                nc.vector.tensor_add(attn[:, :cw], attn[:, :cw], tmp[:, :cw])
                        nc.vector.tensor_add(toks[a][:, s0:s0 + cw],
                                             toks[a][:, s0:s0 + cw], attn[:, :cw])
                # LN2 + FFN (writes hf in place)
                ln_allreduce(toks, count1)
                for s0, cw in CS:
                    psh = psB.tile([128, 512], F32, tag="bc", bufs=2)
                    first_mm = True
                    for a in range(KP1):
                        xa = wp.tile([128, 512], F32, tag="xa", bufs=2)
                        nc.vector.tensor_scalar(
                            out=xa[:, :cw], in0=toks[a][:, s0:s0 + cw],
                            scalar1=ab128[:, 0:1], scalar2=ab128[:, 1:2],
                            op0=OP.mult, op1=OP.add)
                        for m in range(4):
                            psf = psA.tile([128, 512], F32, tag="mm")
                            nc.tensor.matmul(out=psf[:, :cw], lhsT=W(f'f1W{m}'),
                                             rhs=xa[:, :cw], start=True, stop=True)
                            rl = wp.tile([128, 512], F32, tag="rl", bufs=2)
                            nc.scalar.activation(out=rl[:, :cw], in_=psf[:, :cw],
                                                 func=AF.Lrelu, bias=W(f'f1b{m}'),
                                                 alpha=LRELU_ALPHA)
                            nc.tensor.matmul(out=psh[:, :cw], lhsT=W(f'f2W{m}'),
                                             rhs=rl[:, :cw], start=first_mm,
                                             stop=(a == KP1 - 1 and m == 3))
                            first_mm = False
                    nc.scalar.activation(out=hf[:, s0:s0 + cw], in_=psh[:, :cw],
                                         func=AF.Identity, bias=W('f2b3'))

            # ---------------- final LN + decoder + log_softmax ----------------
            ln_allreduce([hf], count3)
            nc.vector.tensor_scalar(out=hf[:, :S], in0=hf[:, :S],
                                    scalar1=ab128[:, 0:1], scalar2=ab128[:, 1:2],
                                    op0=OP.mult, op1=OP.add)
            for s0, cw in CS:
                psd = psA.tile([128, 512], F32, tag="mm")
                nc.tensor.matmul(out=psd[0:Cc, :cw], lhsT=W('decW'),
                                 rhs=hf[:, s0:s0 + cw], start=True, stop=True)
                dsb = wp.tile([Cc, 512], F32, tag="dsb", bufs=2)
                nc.scalar.activation(out=dsb[:, :cw], in_=psd[0:Cc, :cw],
                                     func=AF.Identity, bias=Wr('decb', Cc))
                b0 = 0
                while b0 < cw:
                    bw = min(128, cw - b0)
                    pst = psA.tile([128, 128], F32, tag="tp")
                    nc.tensor.transpose(out=pst[0:bw, 0:Cc], in_=dsb[:, b0:b0 + bw],
                                        identity=eye[0:Cc, 0:Cc])
                    lg = wp.tile([128, Cc], F32, tag="lg", bufs=2)
                    nc.scalar.activation(out=lg[0:bw, :], in_=pst[0:bw, 0:Cc], func=AF.Copy)
                    mx = wp.tile([128, 2], F32, tag="mx", bufs=2)
                    nc.vector.tensor_reduce(out=mx[0:bw, 0:1], in_=lg[0:bw, :],
                                            axis=mybir.AxisListType.X, op=OP.max,
                                            negate=True)
                    ex = wp.tile([128, Cc], F32, tag="ex", bufs=2)
                    sume = wp.tile([128, 2], F32, tag="sume", bufs=2)
                    nc.scalar.activation(out=ex[0:bw, :], in_=lg[0:bw, :], func=AF.Exp,
                                         bias=mx[0:bw, 0:1], accum_out=sume[0:bw, 0:1])
                    lnz = wp.tile([128, 2], F32, tag="lnz", bufs=2)
                    nc.scalar.activation(out=lnz[0:bw, 0:1], in_=sume[0:bw, 0:1],
                                         func=AF.Ln, bias=Wr('zero1', bw))
                    fin = wp.tile([128, Cc], F32, tag="fin", bufs=2)
                    nc.vector.tensor_scalar(
                        out=fin[0:bw, :], in0=lg[0:bw, :],
                        scalar1=mx[0:bw, 0:1], scalar2=lnz[0:bw, 0:1],
                        op0=OP.add, op1=OP.subtract)
                    nc.sync.dma_start(out_t.ap()[s0 + b0:s0 + b0 + bw, :], fin[0:bw, :])
                    b0 += bw

    nc.compile()
    return nc


# ------------------------------------------------------------------- driver

_CACHE = {}


def run(x, edge_index, params, NL):
    x = np.asarray(x, np.float32)
    edge_index = np.asarray(edge_index)
    N = x.shape[0]
    pre = _preprocess(edge_index, N)
    w, lpacks, FP = _pack_weights(params, NL)
    wts_np = w.tensor()
    lwts_np = np.stack([lp.tensor() for lp in lpacks], axis=0)
    C = np.asarray(params['dec_W']).shape[1]
    cfg = dict(N=N, NL=NL, C=C, S=pre['S'], SP=pre['SP'], CH=pre['CH'],
               NCALL=pre['NCALL'], FP=FP, NW=wts_np.shape[1],
               NWL=lwts_np.shape[2], wpos=w.pos, lwpos=lpacks[0].pos,
               sched=pre['sched'])
    key = (N, NL, C, pre['NCALL'], FP, wts_np.shape[1], lwts_np.shape[2],
           tuple(pre['sched'][:8]))
    if key not in _CACHE:
        _CACHE[key] = _build(cfg)
    nc = _CACHE[key]

    S = pre['S']
    in_maps = []
    for c in range(NCORES):
        xp = np.zeros((FP, S), np.float32)
        xp[:x.shape[1], :] = x[pre['perms'][c]].T
        in_maps.append({"xT": xp, "offs": pre['offs'][c],
                        "pvec": pre['pvecs'][c], "wts": wts_np,
                        "lwts": lwts_np})
    res = bass_utils.run_bass_kernel_spmd(nc, in_maps, core_ids=list(range(NCORES)),
                                          trace=os.environ.get('KTRACE', '0') == '1')
    out = np.empty((N, C), np.float32)
    for c in range(NCORES):
        out[pre['perms'][c]] = res.results[c]['out']
    run.last_exec_time_ns = res.exec_time_ns
    return out


run.last_exec_time_ns = None


def kernel(x, edge_index, params):
    return run(x, edge_index, params, NL=len(params['layers']))
